# revision 3
# baseline (speedup 1.0000x reference)
"""Trainium2 Bass kernel for CrossMultiHeadedSelfAttention.

Problem: B=2, SQ=SK=2048, D=1024, H=16, HD=64 cross-attention
  q = x @ Wq + bq ; k = enc @ Wk + bk ; v = enc @ Wv + bv   (per head)
  out = softmax(q k^T / sqrt(HD)) v  -> concat heads -> @ Wo + bo

Sharding: 8 cores = 2 batches x 4 head-groups (4 heads per core).
Each core computes a partial output projection over its 4 heads; the host
sums the 4 partials per batch and adds the constant term
(bo + sum_h bv_h @ Wo_h, exact because softmax rows sum to 1).

Device-side math (per core, bf16 matmuls, f32 accumulation):
  - x/enc are pre-transposed AND pre-cast to bf16 on the host, so xT/encT
    d-major tiles load with fully contiguous DMA
  - qT/kT in [head-pair e (128) x seq] layout, bias via per-partition
    tensor_scalar add; v in natural [s, 4*65] layout with a ones column
    per head (gives softmax row-sums for free in the attn@v matmul)
  - scoresT chunk: the two heads of a pair are row-tile-packed on the PE
    (tile_position (0,0)/(64,0)) and run concurrently
  - exp is split between the scalar engine (table exp) and the vector
    engine (custom 2-op chain: cubic seed of e^{x/64} then 6 squarings,
    rel err ~1e-5) so the exp-bound inner loop is paced by neither alone
  - outU = v'_h.T @ expT  ([65 x 512] in PSUM, row 64 = softmax row-sum)
  - normalize without any transpose: reciprocal of row 64 -> tiny
    partition-broadcast DMA [1,512]->[64,512] -> one tensor_mul writes the
    normalized e-major tile into the pair-stacked stk buffer
  - y = sum_pairs stk_pair.T @ Wo_pair  (K=128); the 4 qt-groups of block
    qb are interjected into block qb+1's kc loop (PE slack there), so the
    normalize->project->DMA chain never stalls the exp pipeline
"""

import sys

for _p in ("/opt/trn_rl_repo", "/root/.axon_site/_ro/trn_rl_repo"):
    if _p not in sys.path:
        sys.path.insert(0, _p)

import numpy as np
import ml_dtypes

import concourse.bass as bass
import concourse.tile as tile
from concourse import bacc, mybir
from concourse.bass_utils import run_bass_kernel_spmd

BF16 = mybir.dt.bfloat16
F32 = mybir.dt.float32
AF = mybir.ActivationFunctionType

B, S, D, H, HD = 2, 2048, 1024, 16, 64
NCORES = 8
HPC = 4          # heads per core
NPAIR = 2        # head pairs per core
DC = D // 128    # 8 d-chunks
KC = S // 128    # 16 k-chunks
NQB = 4          # q blocks of 512
QB = 512
NQT = QB // 128  # q tiles per block

# kc iterations whose h2=1 exp half runs on the vector engine
DVE_KCS = frozenset({2, 5, 8, 11, 14})

_CACHE = {}
_EXP_OPS = {}


def _register_exp_ops():
    """Register two custom DVE ops computing exp(x*scale) as
    (1 + t + t^2/2 + t^3/6)^64 with t = x*scale/64 (rel err ~1.3e-5):
    ANT_EXPP_SEED evaluates the cubic, ANT_EXPP_SQ6 squares six times."""
    if _EXP_OPS:
        return
    import concourse.dve_ops as dops
    from concourse.dve_spec import Spec, Src0, C0, C1, C2, One, sq
    from concourse.dve_spec import lower as dve_lower
    from concourse.dve_uop import DveOpSpec

    t = Src0 * C0
    t2 = t * t
    t3 = t2 * t
    seed_spec = Spec(
        body=((One + t) + t2 * C1) + t3 * C2,
        reference=lambda in0, in1, s0, s1, imm2: (
            1.0 + in0 * s0 + (in0 * s0) ** 2 * s1 + (in0 * s0) ** 3 * imm2
        ).astype(np.float32),
    )
    b = Src0
    for _ in range(6):
        b = sq(b)
    sq6_spec = Spec(
        body=b,
        reference=lambda in0, in1, s0, s1, imm2: (
            in0.astype(np.float64) ** 64
        ).astype(np.float32),
    )

    for name, spec in (("ANT_EXPP_SEED", seed_spec), ("ANT_EXPP_SQ6", sq6_spec)):
        if name not in dops._SUB_OPCODE_FOR_NAME:
            row = max(dops._SUB_OPCODE_FOR_NAME.values()) + 1
            assert row < 0x20, "custom-DVE row field overflow"
            dops._SUB_OPCODE_FOR_NAME[name] = row
        shas = {}
        for ver in ("v3", "v4"):
            uops = dve_lower(spec, ver=ver)
            shas[ver] = DveOpSpec(
                name=name, opcode=dops._SUB_OPCODE_FOR_NAME[name], uops=uops,
                rd1_en=False,
            ).sha(ver)
        op = dops.DveOp(name, spec, subdim=False, uops_sha=shas)
        if all(o.name != name for o in dops.OPS):
            dops.OPS.append(op)
        dops.CUSTOM_DVE_SPECS[name] = spec
        _EXP_OPS[name] = op


def _build_program():
    _register_exp_ops()
    seed_op = _EXP_OPS["ANT_EXPP_SEED"]
    sq6_op = _EXP_OPS["ANT_EXPP_SQ6"]

    nc = bacc.Bacc("TRN2", target_bir_lowering=False, debug=False, num_devices=NCORES)

    xt = nc.dram_tensor("xt", [D, S], BF16, kind="ExternalInput").ap()
    et = nc.dram_tensor("et", [D, S], BF16, kind="ExternalInput").ap()
    wq = nc.dram_tensor("wq", [128, NPAIR, DC, 128], BF16, kind="ExternalInput").ap()
    wk = nc.dram_tensor("wk", [128, NPAIR, DC, 128], BF16, kind="ExternalInput").ap()
    wv = nc.dram_tensor("wv", [128, DC, 256], BF16, kind="ExternalInput").ap()
    wo = nc.dram_tensor("wo", [128, NPAIR, D], BF16, kind="ExternalInput").ap()
    bq = nc.dram_tensor("bq", [128, NPAIR], F32, kind="ExternalInput").ap()
    bk = nc.dram_tensor("bk", [128, NPAIR], F32, kind="ExternalInput").ap()
    out = nc.dram_tensor("out", [S, D], F32, kind="ExternalOutput").ap()

    with tile.TileContext(nc) as tc:
        from contextlib import ExitStack

        with ExitStack() as ctx:
            wts = ctx.enter_context(tc.tile_pool(name="wts", bufs=1))
            big = ctx.enter_context(tc.tile_pool(name="big", bufs=1))

            # weights via gpsimd SWDGE; bulk activations via sync HWDGE
            wq_sb = wts.tile([128, NPAIR, DC, 128], BF16, name="wq_sb")
            wk_sb = wts.tile([128, NPAIR, DC, 128], BF16, name="wk_sb")
            wv_sb = wts.tile([128, DC, 256], BF16, name="wv_sb")
            wo_sb = wts.tile([128, NPAIR, D], BF16, name="wo_sb")
            bq_sb = wts.tile([128, NPAIR], F32, name="bq_sb")
            bk_sb = wts.tile([128, NPAIR], F32, name="bk_sb")
            for sb, dr in ((wq_sb, wq), (wk_sb, wk), (wv_sb, wv), (wo_sb, wo),
                           (bq_sb, bq), (bk_sb, bk)):
                nc.gpsimd.dma_start(sb, dr)

            # column-block loads so the first projection chunk only waits on
            # ~1MB of activations, not the full 8MB
            xT = [big.tile([128, S], BF16, name=f"xT{d}") for d in range(DC)]
            eT = [big.tile([128, S], BF16, name=f"eT{d}") for d in range(DC)]
            for sb4 in range(NQB):
                sl = slice(sb4 * QB, (sb4 + 1) * QB)
                for d in range(DC):
                    nc.sync.dma_start(eT[d][:, sl], et[d * 128:(d + 1) * 128, sl])
                if sb4 == 0:
                    for d in range(DC):
                        nc.sync.dma_start(xT[d][:, sl],
                                          xt[d * 128:(d + 1) * 128, sl])
            for sb4 in range(1, NQB):
                sl = slice(sb4 * QB, (sb4 + 1) * QB)
                for d in range(DC):
                    nc.sync.dma_start(xT[d][:, sl], xt[d * 128:(d + 1) * 128, sl])

            # ---- unified PSUM pools (8 banks total, live for whole kernel) ----
            dpool = ctx.enter_context(tc.tile_pool(name="dpool", bufs=4,
                                                   space="DRAM"))
            psc = ctx.enter_context(tc.tile_pool(name="psc", bufs=2, space="PSUM"))
            pou = ctx.enter_context(tc.tile_pool(name="pou", bufs=2, space="PSUM"))
            py = ctx.enter_context(tc.tile_pool(name="py", bufs=2, space="PSUM"))
            wk2 = ctx.enter_context(tc.tile_pool(name="wk2", bufs=2))
            expp = ctx.enter_context(tc.tile_pool(name="expp", bufs=6))
            exps = ctx.enter_context(tc.tile_pool(name="exps", bufs=2))

            # ---- projections; only kT[0] + v gate the first attention ----
            qT = [big.tile([128, S], BF16, name=f"qT{p}") for p in range(NPAIR)]
            kT = [big.tile([128, S], BF16, name=f"kT{p}") for p in range(NPAIR)]
            v = [big.tile([128, HPC, 65], BF16, name=f"v{s}") for s in range(KC)]

            def project_k_chunk(p, sb4):
                # deferred projections use the y-slot (idle during attention)
                sl = slice(sb4 * QB, (sb4 + 1) * QB)
                pk = py.tile([128, QB], F32, name="pk", tag="y")
                for d in range(DC):
                    nc.tensor.matmul(pk, wk_sb[:, p, d, :], eT[d][:, sl],
                                     start=(d == 0), stop=(d == DC - 1))
                nc.vector.tensor_scalar_add(kT[p][:, sl], pk, bk_sb[:, p:p + 1])

            def project_q(p, qb, tag="sc"):
                qsl = slice(qb * QB, (qb + 1) * QB)
                pool = psc if tag == "sc" else py
                pq = pool.tile([128, QB], F32, name="pq", tag=tag)
                for d in range(DC):
                    nc.tensor.matmul(pq, wq_sb[:, p, d, :], xT[d][:, qsl],
                                     start=(d == 0), stop=(d == DC - 1))
                nc.vector.tensor_scalar_add(qT[p][:, qsl], pq, bq_sb[:, p:p + 1])

            def project_v_tile(s):
                pv = py.tile([128, 256], F32, name="pv", tag="y")
                for d in range(DC):
                    nc.tensor.matmul(pv, eT[d][:, s * 128:(s + 1) * 128],
                                     wv_sb[:, d, :],
                                     start=(d == 0), stop=(d == DC - 1))
                nc.vector.tensor_copy(
                    v[s][:, :, 0:64], pv.rearrange("p (h e) -> p h e", h=HPC))
                nc.vector.memset(v[s][:, :, 64:65], 1.0)

            # out-projection for one 128-row q tile of block qbx; interjected
            # into the following block's kc loop (or emitted directly for the
            # last block)
            def outproj_qt(qbx, qt, stk_pair):
                tsl = slice(qt * 128, (qt + 1) * 128)
                ysb = wk2.tile([128, D], F32, name="ysb", tag="ysb", bufs=2)
                for dc2 in range(2):
                    dsl = slice(dc2 * 512, (dc2 + 1) * 512)
                    yp = py.tile([128, 512], F32, name="yp", tag="y")
                    for p in range(NPAIR):
                        nc.tensor.matmul(yp, stk_pair[p][:, tsl],
                                         wo_sb[:, p, dsl],
                                         start=(p == 0), stop=(p == NPAIR - 1))
                    nc.vector.tensor_copy(ysb[:, dsl], yp)
                nc.sync.dma_start(out[qbx * QB + qt * 128:
                                      qbx * QB + (qt + 1) * 128, :], ysb)

            # minimal prologue: just what (qb0, pair0, kc0) needs; every other
            # projection is interjected into the first q block's attention
            # loops (PE slack there; exp is the pacing engine in the kc loop)
            project_k_chunk(0, 0)
            project_v_tile(0)
            project_q(0, 0)

            def interject_qb0_p0(kc):
                if kc == 0:
                    project_v_tile(1); project_v_tile(2)
                elif kc == 1:
                    project_v_tile(3); project_k_chunk(0, 1)
                elif kc == 2:
                    project_v_tile(4); project_v_tile(5)
                elif kc == 3:
                    project_v_tile(6); project_v_tile(7)
                elif kc == 5:
                    project_k_chunk(0, 2)
                elif kc == 6:
                    project_v_tile(8); project_v_tile(9)
                elif kc == 7:
                    project_v_tile(10); project_v_tile(11)
                elif kc == 9:
                    project_k_chunk(0, 3)
                elif kc == 10:
                    project_v_tile(12); project_v_tile(13)
                elif kc == 11:
                    project_v_tile(14); project_v_tile(15)
                elif kc == 12:
                    project_k_chunk(1, 0)
                elif kc == 13:
                    project_q(1, 0)
                elif kc == 15:
                    project_q(0, 1, tag="y")

            def interject_qb0_p1(kc):
                if kc == 0:
                    project_k_chunk(1, 1)
                elif kc == 4:
                    project_k_chunk(1, 2)
                elif kc == 8:
                    project_k_chunk(1, 3)
                elif kc == 11:
                    project_q(1, 1, tag="y")

            # ---- attention + (deferred) output projection ----
            prev_stk = None  # stk tiles of block qb-1, out-projected during qb
            for qb in range(NQB):
                qsl = slice(qb * QB, (qb + 1) * QB)
                stk = [wk2.tile([128, QB], BF16, name=f"stk{p}", tag=f"stk{p}",
                                bufs=2) for p in range(NPAIR)]
                for p in range(NPAIR):
                    if qb == 0:
                        interject = interject_qb0_p0 if p == 0 else interject_qb0_p1
                    else:
                        def interject(kc, p=p, qb=qb):
                            # previous block's out-projection rides the PE
                            # slack of the p=0 loop; next block's q projection
                            # at kc=11 as before
                            if p == 0 and kc in (2, 4, 6, 8):
                                outproj_qt(qb - 1, (kc - 2) // 2, prev_stk)
                            elif kc == 11 and qb < NQB - 1:
                                project_q(p, qb + 1, tag="y")
                    ou = [pou.tile([65, QB], F32, name=f"ou{h2}", tag="ou")
                          for h2 in range(2)]
                    for kc in range(KC):
                        ksl = slice(kc * 128, (kc + 1) * 128)
                        sc = psc.tile([128, 2, QB], F32, name="sc", tag="sc")
                        ex = expp.tile([128, 2, QB], BF16, name="ex", tag="ex")
                        for h2 in range(2):
                            hp = slice(h2 * 64, (h2 + 1) * 64)
                            nc.tensor.matmul(sc[:, h2, :], kT[p][hp, ksl],
                                             qT[p][hp, qsl])
                        if kc in DVE_KCS:
                            # split: scalar engine exps h2=0, vector engine
                            # exps h2=1 via the custom poly-squaring chain
                            nc.scalar.activation(ex[:, 0, :], sc[:, 0, :],
                                                 AF.Exp, scale=0.125)
                            sd = exps.tile([128, QB], F32, name="sd", tag="sd")
                            nc.vector._custom_dve(
                                seed_op, out=sd, in0=sc[:, 1, :],
                                s0=0.125 / 64.0, s1=0.5, imm2=1.0 / 6.0)
                            nc.vector._custom_dve(
                                sq6_op, out=ex[:, 1, :], in0=sd)
                        else:
                            nc.scalar.activation(ex, sc, AF.Exp, scale=0.125)
                        for h2 in range(2):
                            nc.tensor.matmul(ou[h2], v[kc][:, 2 * p + h2, :],
                                             ex[:, h2, :],
                                             start=(kc == 0), stop=(kc == KC - 1))
                        if interject is not None:
                            interject(kc)
                    for h2 in range(2):
                        # copy PSUM->SBUF promptly so the ou slot frees for the
                        # next pair; normalize off the critical path:
                        # reciprocal of rowsum row -> partition-broadcast via a
                        # DRAM bounce (step-0 partition APs are DRAM-only) ->
                        # one multiply into the pair-stacked e-major tile
                        osb = wk2.tile([65, QB], F32, name=f"osb{h2}",
                                       tag=f"osb{h2}", bufs=2)
                        nc.vector.tensor_copy(osb, ou[h2])
                        rr = wk2.tile([65, QB], F32, name="rr", tag="rr", bufs=4)
                        nc.vector.reciprocal_approx_fast(rr, osb)
                        rrd = dpool.tile([1, QB], F32, name="rrd", tag="rrd")
                        nc.gpsimd.dma_start(rrd, rr[64:65, :])
                        rb = wk2.tile([64, QB], F32, name="rb", tag="rb", bufs=4)
                        rr_bcast = bass.AP(tensor=rrd.tensor, offset=rrd.offset,
                                           ap=[[0, 64]] + list(rrd.ap[1:]))
                        nc.gpsimd.dma_start(rb, rr_bcast)
                        nc.vector.tensor_mul(stk[p][h2 * 64:(h2 + 1) * 64, :],
                                             osb[0:64, :], rb)
                prev_stk = stk
            # final block's out-projection has no following loop to hide in
            for qt in range(NQT):
                outproj_qt(NQB - 1, qt, prev_stk)

    nc.compile()
    return nc


def _bf16(a):
    return np.ascontiguousarray(a.astype(ml_dtypes.bfloat16))


def _host_prep(inputs):
    x = np.asarray(inputs["x"], np.float32)
    enc = np.asarray(inputs["encoder_output"], np.float32)
    Wq = np.asarray(inputs["Wq"], np.float32)
    bq = np.asarray(inputs["bq"], np.float32)
    Wk = np.asarray(inputs["Wk"], np.float32)
    bk = np.asarray(inputs["bk"], np.float32)
    Wv = np.asarray(inputs["Wv"], np.float32)
    Wo = np.asarray(inputs["Wo"], np.float32)

    xt_b = [_bf16(x[b].T) for b in range(B)]
    et_b = [_bf16(enc[b].T) for b in range(B)]

    in_maps = []
    for c in range(NCORES):
        b = c // 4
        hb = HPC * (c % 4)

        wq_c = Wq[hb:hb + 4].reshape(2, 2, DC, 128, HD)  # [pair, hw, dc, dp, e]
        wq_c = wq_c.transpose(3, 0, 2, 1, 4).reshape(128, NPAIR, DC, 128)
        wk_c = Wk[hb:hb + 4].reshape(2, 2, DC, 128, HD)
        wk_c = wk_c.transpose(3, 0, 2, 1, 4).reshape(128, NPAIR, DC, 128)
        wv_c = Wv[hb:hb + 4].reshape(4, DC, 128, HD)
        wv_c = wv_c.transpose(2, 1, 0, 3).reshape(128, DC, 256)
        wo_c = Wo[hb * HD:(hb + 4) * HD].reshape(2, 2, HD, D)  # [pair, hw, e, d]
        wo_c = wo_c.transpose(1, 2, 0, 3).reshape(128, NPAIR, D)
        bq_c = bq[hb:hb + 4].reshape(2, 2, HD).transpose(1, 2, 0).reshape(128, NPAIR)
        bk_c = bk[hb:hb + 4].reshape(2, 2, HD).transpose(1, 2, 0).reshape(128, NPAIR)

        in_maps.append({
            "xt": xt_b[b],
            "et": et_b[b],
            "wq": _bf16(wq_c),
            "wk": _bf16(wk_c),
            "wv": _bf16(wv_c),
            "wo": _bf16(wo_c),
            "bq": np.ascontiguousarray(bq_c),
            "bk": np.ascontiguousarray(bk_c),
        })
    return in_maps


def kernel(**inputs):
    if "nc" not in _CACHE:
        _CACHE["nc"] = _build_program()
    nc = _CACHE["nc"]

    in_maps = _host_prep(inputs)
    res = None
    for attempt in range(3):
        try:
            res = run_bass_kernel_spmd(nc, in_maps, core_ids=list(range(NCORES)))
            break
        except Exception:
            if attempt == 2:
                raise
            import time
            time.sleep(5)
    _CACHE["last_results"] = res

    bv = np.asarray(inputs["bv"], np.float32)
    Wo = np.asarray(inputs["Wo"], np.float32)
    bo = np.asarray(inputs["bo"], np.float32)
    const_d = bo + np.einsum("he,hed->d", bv,
                             Wo.reshape(H, HD, D)).astype(np.float32)

    out = np.empty((B, S, D), np.float32)
    for b in range(B):
        acc = res.results[4 * b]["out"].astype(np.float32).copy()
        for c in range(4 * b + 1, 4 * b + 4):
            acc += res.results[c]["out"]
        out[b] = acc + const_d
    return out


# revision 7
# speedup vs baseline: 1.1045x; 1.1045x over previous
"""Trainium2 Bass kernel for CrossMultiHeadedSelfAttention.

Problem: B=2, SQ=SK=2048, D=1024, H=16, HD=64 cross-attention
  q = x @ Wq + bq ; k = enc @ Wk + bk ; v = enc @ Wv + bv   (per head)
  out = softmax(q k^T / sqrt(HD)) v  -> concat heads -> @ Wo + bo

Sharding: 8 cores = 2 batches x 4 head-groups (4 heads per core).
Each core computes a partial output projection over its 4 heads; the host
sums the 4 partials per batch and adds the constant term
(bo + sum_h bv_h @ Wo_h, exact because softmax rows sum to 1).

Device-side math (per core, bf16 matmuls, f32 accumulation):
  - x/enc are pre-transposed AND pre-cast to bf16 on the host, so xT/encT
    d-major tiles load with fully contiguous DMA
  - qT/kT in [head-pair e (128) x seq] layout, bias via per-partition
    tensor_scalar add; v in natural [s, 4*65] layout with a ones column
    per head (gives softmax row-sums for free in the attn@v matmul)
  - scoresT chunk: the two heads of a pair are row-tile-packed on the PE
    (tile_position (0,0)/(64,0)) and run concurrently
  - exp is split between the scalar engine (table exp) and the vector
    engine (custom 2-op chain: cubic seed of e^{x/64} then 6 squarings,
    rel err ~1e-5) so the exp-bound inner loop is paced by neither alone
  - outU = v'_h.T @ expT  ([65 x 512] in PSUM, row 64 = softmax row-sum)
  - normalize without any transpose: reciprocal of row 64 -> tiny
    partition-broadcast DMA [1,512]->[64,512] -> one tensor_mul writes the
    normalized e-major tile into the pair-stacked stk buffer
  - y = sum_pairs stk_pair.T @ Wo_pair  (K=128); the 4 qt-groups of block
    qb are interjected into block qb+1's kc loop (PE slack there), so the
    normalize->project->DMA chain never stalls the exp pipeline
"""

import sys

for _p in ("/opt/trn_rl_repo", "/root/.axon_site/_ro/trn_rl_repo"):
    if _p not in sys.path:
        sys.path.insert(0, _p)

import numpy as np
import ml_dtypes

import concourse.bass as bass
import concourse.tile as tile
from concourse import bacc, mybir
from concourse.bass_utils import run_bass_kernel_spmd

BF16 = mybir.dt.bfloat16
F32 = mybir.dt.float32
AF = mybir.ActivationFunctionType

B, S, D, H, HD = 2, 2048, 1024, 16, 64
NCORES = 8
HPC = 4          # heads per core
NPAIR = 2        # head pairs per core
DC = D // 128    # 8 d-chunks
KC = S // 128    # 16 k-chunks
NQB = 4          # q blocks of 512
QB = 512
NQT = QB // 128  # q tiles per block

# kc iterations whose h2=1 exp half runs on the vector engine (must avoid
# 0, 14 and 15: the deferred attn@v matmul would break the PSUM
# accumulation-group start/stop ordering)
DVE_KCS = frozenset({2, 4, 7, 10, 13})

_CACHE = {}
_EXP_OPS = {}


def _register_exp_ops():
    """Register two custom DVE ops computing exp(x*scale) as
    (1 + t + t^2/2 + t^3/6)^64 with t = x*scale/64 (rel err ~1.3e-5):
    ANT_EXPP_SEED evaluates the cubic, ANT_EXPP_SQ6 squares six times."""
    if _EXP_OPS:
        return
    import concourse.dve_ops as dops
    from concourse.dve_spec import Spec, Src0, C0, C1, C2, One, sq
    from concourse.dve_spec import lower as dve_lower
    from concourse.dve_uop import DveOpSpec

    t = Src0 * C0
    t2 = t * t
    t3 = t2 * t
    seed_spec = Spec(
        body=((One + t) + t2 * C1) + t3 * C2,
        reference=lambda in0, in1, s0, s1, imm2: (
            1.0 + in0 * s0 + (in0 * s0) ** 2 * s1 + (in0 * s0) ** 3 * imm2
        ).astype(np.float32),
    )
    b = Src0
    for _ in range(6):
        b = sq(b)
    sq6_spec = Spec(
        body=b,
        reference=lambda in0, in1, s0, s1, imm2: (
            in0.astype(np.float64) ** 64
        ).astype(np.float32),
    )

    for name, spec in (("ANT_EXPP_SEED", seed_spec), ("ANT_EXPP_SQ6", sq6_spec)):
        if name not in dops._SUB_OPCODE_FOR_NAME:
            row = max(dops._SUB_OPCODE_FOR_NAME.values()) + 1
            assert row < 0x20, "custom-DVE row field overflow"
            dops._SUB_OPCODE_FOR_NAME[name] = row
        shas = {}
        for ver in ("v3", "v4"):
            uops = dve_lower(spec, ver=ver)
            shas[ver] = DveOpSpec(
                name=name, opcode=dops._SUB_OPCODE_FOR_NAME[name], uops=uops,
                rd1_en=False,
            ).sha(ver)
        op = dops.DveOp(name, spec, subdim=False, uops_sha=shas)
        if all(o.name != name for o in dops.OPS):
            dops.OPS.append(op)
        dops.CUSTOM_DVE_SPECS[name] = spec
        _EXP_OPS[name] = op


def _build_program():
    _register_exp_ops()
    seed_op = _EXP_OPS["ANT_EXPP_SEED"]
    sq6_op = _EXP_OPS["ANT_EXPP_SQ6"]

    nc = bacc.Bacc("TRN2", target_bir_lowering=False, debug=False, num_devices=NCORES)

    xt = nc.dram_tensor("xt", [D, S], BF16, kind="ExternalInput").ap()
    et = nc.dram_tensor("et", [D, S], BF16, kind="ExternalInput").ap()
    wq = nc.dram_tensor("wq", [128, NPAIR, DC, 128], BF16, kind="ExternalInput").ap()
    wk = nc.dram_tensor("wk", [128, NPAIR, DC, 128], BF16, kind="ExternalInput").ap()
    wv = nc.dram_tensor("wv", [128, DC, 256], BF16, kind="ExternalInput").ap()
    wo = nc.dram_tensor("wo", [128, NPAIR, D], BF16, kind="ExternalInput").ap()
    bq = nc.dram_tensor("bq", [128, NPAIR], F32, kind="ExternalInput").ap()
    bk = nc.dram_tensor("bk", [128, NPAIR], F32, kind="ExternalInput").ap()
    out = nc.dram_tensor("out", [S, D], F32, kind="ExternalOutput").ap()

    with tile.TileContext(nc) as tc:
        from contextlib import ExitStack

        with ExitStack() as ctx:
            wts = ctx.enter_context(tc.tile_pool(name="wts", bufs=1))
            big = ctx.enter_context(tc.tile_pool(name="big", bufs=1))

            # weights via gpsimd SWDGE; bulk activations via sync HWDGE
            wq_sb = wts.tile([128, NPAIR, DC, 128], BF16, name="wq_sb")
            wk_sb = wts.tile([128, NPAIR, DC, 128], BF16, name="wk_sb")
            wv_sb = wts.tile([128, DC, 256], BF16, name="wv_sb")
            wo_sb = wts.tile([128, NPAIR, D], BF16, name="wo_sb")
            bq_sb = wts.tile([128, NPAIR], F32, name="bq_sb")
            bk_sb = wts.tile([128, NPAIR], F32, name="bk_sb")
            for sb, dr in ((wq_sb, wq), (wk_sb, wk), (wv_sb, wv), (wo_sb, wo),
                           (bq_sb, bq), (bk_sb, bk)):
                nc.gpsimd.dma_start(sb, dr)

            # column-block loads so the first projection chunk only waits on
            # ~1MB of activations, not the full 8MB
            xT = [big.tile([128, S], BF16, name=f"xT{d}") for d in range(DC)]
            eT = [big.tile([128, S], BF16, name=f"eT{d}") for d in range(DC)]
            for sb4 in range(NQB):
                sl = slice(sb4 * QB, (sb4 + 1) * QB)
                for d in range(DC):
                    nc.sync.dma_start(eT[d][:, sl], et[d * 128:(d + 1) * 128, sl])
                if sb4 == 0:
                    for d in range(DC):
                        nc.sync.dma_start(xT[d][:, sl],
                                          xt[d * 128:(d + 1) * 128, sl])
            for sb4 in range(1, NQB):
                sl = slice(sb4 * QB, (sb4 + 1) * QB)
                for d in range(DC):
                    nc.sync.dma_start(xT[d][:, sl], xt[d * 128:(d + 1) * 128, sl])

            # ---- unified PSUM pools (8 banks total, live for whole kernel) ----
            dpool = ctx.enter_context(tc.tile_pool(name="dpool", bufs=4,
                                                   space="DRAM"))
            psc = ctx.enter_context(tc.tile_pool(name="psc", bufs=2, space="PSUM"))
            pou = ctx.enter_context(tc.tile_pool(name="pou", bufs=2, space="PSUM"))
            py = ctx.enter_context(tc.tile_pool(name="py", bufs=2, space="PSUM"))
            wk2 = ctx.enter_context(tc.tile_pool(name="wk2", bufs=2))
            expp = ctx.enter_context(tc.tile_pool(name="expp", bufs=6))
            exps = ctx.enter_context(tc.tile_pool(name="exps", bufs=2))

            # ---- projections; only kT[0] + v gate the first attention ----
            qT = [big.tile([128, S], BF16, name=f"qT{p}") for p in range(NPAIR)]
            kT = [big.tile([128, S], BF16, name=f"kT{p}") for p in range(NPAIR)]
            v = [big.tile([128, HPC, 65], BF16, name=f"v{s}") for s in range(KC)]

            def project_k_chunk(p, sb4):
                # deferred projections use the y-slot (idle during attention)
                sl = slice(sb4 * QB, (sb4 + 1) * QB)
                pk = py.tile([128, QB], F32, name="pk", tag="y")
                for d in range(DC):
                    nc.tensor.matmul(pk, wk_sb[:, p, d, :], eT[d][:, sl],
                                     start=(d == 0), stop=(d == DC - 1))
                nc.vector.tensor_scalar_add(kT[p][:, sl], pk, bk_sb[:, p:p + 1])

            def project_q(p, qb, tag="sc"):
                qsl = slice(qb * QB, (qb + 1) * QB)
                pool = psc if tag == "sc" else py
                pq = pool.tile([128, QB], F32, name="pq", tag=tag)
                for d in range(DC):
                    nc.tensor.matmul(pq, wq_sb[:, p, d, :], xT[d][:, qsl],
                                     start=(d == 0), stop=(d == DC - 1))
                nc.vector.tensor_scalar_add(qT[p][:, qsl], pq, bq_sb[:, p:p + 1])

            def project_v_tile(s):
                pv = py.tile([128, 256], F32, name="pv", tag="y")
                for d in range(DC):
                    nc.tensor.matmul(pv, eT[d][:, s * 128:(s + 1) * 128],
                                     wv_sb[:, d, :],
                                     start=(d == 0), stop=(d == DC - 1))
                nc.vector.tensor_copy(
                    v[s][:, :, 0:64], pv.rearrange("p (h e) -> p h e", h=HPC))
                nc.gpsimd.memset(v[s][:, :, 64:65], 1.0)

            # out-projection for one 128-row q tile of block qbx; interjected
            # into the following block's kc loop (or emitted directly for the
            # last block)
            def outproj_qt(qbx, qt, stk_pair):
                tsl = slice(qt * 128, (qt + 1) * 128)
                ysb = wk2.tile([128, D], F32, name="ysb", tag="ysb", bufs=2)
                for dc2 in range(2):
                    dsl = slice(dc2 * 512, (dc2 + 1) * 512)
                    yp = py.tile([128, 512], F32, name="yp", tag="y")
                    for p in range(NPAIR):
                        nc.tensor.matmul(yp, stk_pair[p][:, tsl],
                                         wo_sb[:, p, dsl],
                                         start=(p == 0), stop=(p == NPAIR - 1))
                    nc.vector.tensor_copy(ysb[:, dsl], yp)
                nc.sync.dma_start(out[qbx * QB + qt * 128:
                                      qbx * QB + (qt + 1) * 128, :], ysb)

            # minimal prologue: just what (qb0, pair0, kc0) needs; every other
            # projection is interjected into the first q block's attention
            # loops (PE slack there; exp is the pacing engine in the kc loop)
            project_k_chunk(0, 0)
            project_v_tile(0)
            project_q(0, 0)

            def interject_qb0_p0(kc):
                if kc == 0:
                    project_v_tile(1); project_v_tile(2)
                elif kc == 1:
                    project_v_tile(3); project_k_chunk(0, 1)
                elif kc == 2:
                    project_v_tile(4); project_v_tile(5)
                elif kc == 3:
                    project_v_tile(6); project_v_tile(7)
                elif kc == 5:
                    project_k_chunk(0, 2)
                elif kc == 6:
                    project_v_tile(8); project_v_tile(9)
                elif kc == 7:
                    project_v_tile(10); project_v_tile(11)
                elif kc == 9:
                    project_k_chunk(0, 3)
                elif kc == 10:
                    project_v_tile(12); project_v_tile(13)
                elif kc == 11:
                    project_v_tile(14); project_v_tile(15)
                elif kc == 12:
                    project_k_chunk(1, 0)
                elif kc == 13:
                    project_q(1, 0)
                elif kc == 15:
                    project_q(0, 1, tag="y")

            def interject_qb0_p1(kc):
                if kc == 0:
                    project_k_chunk(1, 1)
                elif kc == 4:
                    project_k_chunk(1, 2)
                elif kc == 8:
                    project_k_chunk(1, 3)
                elif kc == 11:
                    project_q(1, 1, tag="y")

            # ---- attention + (deferred) output projection ----
            prev_stk = None  # stk tiles of block qb-1, out-projected during qb
            for qb in range(NQB):
                qsl = slice(qb * QB, (qb + 1) * QB)
                stk = [wk2.tile([128, QB], BF16, name=f"stk{p}", tag=f"stk{p}",
                                bufs=2) for p in range(NPAIR)]
                for p in range(NPAIR):
                    if qb == 0:
                        interject = interject_qb0_p0 if p == 0 else interject_qb0_p1
                    else:
                        def interject(kc, p=p, qb=qb):
                            # previous block's out-projection rides the PE
                            # slack of the p=0 loop; next block's q projection
                            # at kc=11 as before
                            if p == 0 and kc in (2, 4, 6, 8):
                                outproj_qt(qb - 1, (kc - 2) // 2, prev_stk)
                            elif kc == 11 and qb < NQB - 1:
                                project_q(p, qb + 1, tag="y")
                    ou = [pou.tile([65, QB], F32, name=f"ou{h2}", tag="ou")
                          for h2 in range(2)]
                    pending = None  # deferred h2=1 attn@v of a DVE-exp kc
                    for kc in range(KC):
                        ksl = slice(kc * 128, (kc + 1) * 128)
                        sc = psc.tile([128, 2, QB], F32, name="sc", tag="sc")
                        ex = expp.tile([128, 2, QB], BF16, name="ex", tag="ex")
                        for h2 in range(2):
                            hp = slice(h2 * 64, (h2 + 1) * 64)
                            nc.tensor.matmul(sc[:, h2, :], kT[p][hp, ksl],
                                             qT[p][hp, qsl])
                        if kc in DVE_KCS:
                            # split: scalar engine exps h2=0, vector engine
                            # exps h2=1 via the custom poly-squaring chain
                            nc.scalar.activation(ex[:, 0, :], sc[:, 0, :],
                                                 AF.Exp, scale=0.125)
                            sd = exps.tile([128, QB], F32, name="sd", tag="sd")
                            nc.vector._custom_dve(
                                seed_op, out=sd, in0=sc[:, 1, :],
                                s0=0.125 / 64.0, s1=0.5, imm2=1.0 / 6.0)
                            nc.vector._custom_dve(
                                sq6_op, out=ex[:, 1, :], in0=sd)
                        else:
                            nc.scalar.activation(ex, sc, AF.Exp, scale=0.125)
                        # flush the previous DVE kc's h2=1 attn@v here, AFTER
                        # this kc's scores: the PE FIFO would otherwise
                        # head-of-line block on the slower DVE exp chain
                        if pending is not None:
                            pkc, pex = pending
                            pending = None
                            nc.tensor.matmul(ou[1], v[pkc][:, 2 * p + 1, :],
                                             pex[:, 1, :],
                                             start=(pkc == 0),
                                             stop=(pkc == KC - 1))
                        nc.tensor.matmul(ou[0], v[kc][:, 2 * p, :],
                                         ex[:, 0, :],
                                         start=(kc == 0), stop=(kc == KC - 1))
                        if kc in DVE_KCS:
                            pending = (kc, ex)
                        else:
                            nc.tensor.matmul(ou[1], v[kc][:, 2 * p + 1, :],
                                             ex[:, 1, :],
                                             start=(kc == 0), stop=(kc == KC - 1))
                        if interject is not None:
                            interject(kc)
                    assert pending is None
                    for h2 in range(2):
                        # copy PSUM->SBUF promptly so the ou slot frees for the
                        # next pair; normalize off the critical path:
                        # reciprocal of rowsum row -> partition-broadcast via a
                        # DRAM bounce (step-0 partition APs are DRAM-only) ->
                        # one multiply into the pair-stacked e-major tile
                        osb = wk2.tile([65, QB], F32, name=f"osb{h2}",
                                       tag=f"osb{h2}", bufs=2)
                        nc.vector.tensor_copy(osb, ou[h2])
                        rr = wk2.tile([65, QB], F32, name="rr", tag="rr", bufs=4)
                        nc.vector.reciprocal_approx_fast(rr, osb)
                        rrd = dpool.tile([1, QB], F32, name="rrd", tag="rrd")
                        nc.gpsimd.dma_start(rrd, rr[64:65, :])
                        rb = wk2.tile([64, QB], F32, name="rb", tag="rb", bufs=4)
                        rr_bcast = bass.AP(tensor=rrd.tensor, offset=rrd.offset,
                                           ap=[[0, 64]] + list(rrd.ap[1:]))
                        nc.gpsimd.dma_start(rb, rr_bcast)
                        nc.gpsimd.tensor_mul(stk[p][h2 * 64:(h2 + 1) * 64, :],
                                             osb[0:64, :], rb)
                prev_stk = stk
            # final block's out-projection has no following loop to hide in
            for qt in range(NQT):
                outproj_qt(NQB - 1, qt, prev_stk)

    nc.compile()
    return nc


def _bf16(a):
    return np.ascontiguousarray(a.astype(ml_dtypes.bfloat16))


def _host_prep(inputs):
    x = np.asarray(inputs["x"], np.float32)
    enc = np.asarray(inputs["encoder_output"], np.float32)
    Wq = np.asarray(inputs["Wq"], np.float32)
    bq = np.asarray(inputs["bq"], np.float32)
    Wk = np.asarray(inputs["Wk"], np.float32)
    bk = np.asarray(inputs["bk"], np.float32)
    Wv = np.asarray(inputs["Wv"], np.float32)
    Wo = np.asarray(inputs["Wo"], np.float32)

    xt_b = [_bf16(x[b].T) for b in range(B)]
    et_b = [_bf16(enc[b].T) for b in range(B)]

    in_maps = []
    for c in range(NCORES):
        b = c // 4
        hb = HPC * (c % 4)

        wq_c = Wq[hb:hb + 4].reshape(2, 2, DC, 128, HD)  # [pair, hw, dc, dp, e]
        wq_c = wq_c.transpose(3, 0, 2, 1, 4).reshape(128, NPAIR, DC, 128)
        wk_c = Wk[hb:hb + 4].reshape(2, 2, DC, 128, HD)
        wk_c = wk_c.transpose(3, 0, 2, 1, 4).reshape(128, NPAIR, DC, 128)
        wv_c = Wv[hb:hb + 4].reshape(4, DC, 128, HD)
        wv_c = wv_c.transpose(2, 1, 0, 3).reshape(128, DC, 256)
        wo_c = Wo[hb * HD:(hb + 4) * HD].reshape(2, 2, HD, D)  # [pair, hw, e, d]
        wo_c = wo_c.transpose(1, 2, 0, 3).reshape(128, NPAIR, D)
        bq_c = bq[hb:hb + 4].reshape(2, 2, HD).transpose(1, 2, 0).reshape(128, NPAIR)
        bk_c = bk[hb:hb + 4].reshape(2, 2, HD).transpose(1, 2, 0).reshape(128, NPAIR)

        in_maps.append({
            "xt": xt_b[b],
            "et": et_b[b],
            "wq": _bf16(wq_c),
            "wk": _bf16(wk_c),
            "wv": _bf16(wv_c),
            "wo": _bf16(wo_c),
            "bq": np.ascontiguousarray(bq_c),
            "bk": np.ascontiguousarray(bk_c),
        })
    return in_maps


def kernel(**inputs):
    if "nc" not in _CACHE:
        _CACHE["nc"] = _build_program()
    nc = _CACHE["nc"]

    in_maps = _host_prep(inputs)
    res = None
    for attempt in range(3):
        try:
            res = run_bass_kernel_spmd(nc, in_maps, core_ids=list(range(NCORES)))
            break
        except Exception:
            if attempt == 2:
                raise
            import time
            time.sleep(5)
    _CACHE["last_results"] = res

    bv = np.asarray(inputs["bv"], np.float32)
    Wo = np.asarray(inputs["Wo"], np.float32)
    bo = np.asarray(inputs["bo"], np.float32)
    const_d = bo + np.einsum("he,hed->d", bv,
                             Wo.reshape(H, HD, D)).astype(np.float32)

    out = np.empty((B, S, D), np.float32)
    for b in range(B):
        acc = res.results[4 * b]["out"].astype(np.float32).copy()
        for c in range(4 * b + 1, 4 * b + 4):
            acc += res.results[c]["out"]
        out[b] = acc + const_d
    return out


# revision 15
# speedup vs baseline: 1.1256x; 1.0191x over previous
"""Trainium2 Bass kernel for CrossMultiHeadedSelfAttention.

Problem: B=2, SQ=SK=2048, D=1024, H=16, HD=64 cross-attention
  q = x @ Wq + bq ; k = enc @ Wk + bk ; v = enc @ Wv + bv   (per head)
  out = softmax(q k^T / sqrt(HD)) v  -> concat heads -> @ Wo + bo

Sharding: 8 cores = 2 batches x 4 head-groups (4 heads per core).
Each core computes a partial output projection over its 4 heads; the host
sums the 4 partials per batch and adds the constant term
(bo + sum_h bv_h @ Wo_h, exact because softmax rows sum to 1).

Device-side math (per core, bf16 matmuls, f32 accumulation):
  - x/enc are pre-transposed AND pre-cast to bf16 on the host, so xT/encT
    d-major tiles load with fully contiguous DMA
  - qT/kT in [head-pair e (128) x seq] layout, bias via per-partition
    tensor_scalar add; v in natural [s, 4*65] layout with a ones column
    per head (gives softmax row-sums for free in the attn@v matmul)
  - scoresT chunk: the two heads of a pair are row-tile-packed on the PE
    (tile_position (0,0)/(64,0)) and run concurrently
  - exp is split between the scalar engine (table exp) and the vector
    engine (custom 2-op chain: cubic seed of e^{x/64} then 6 squarings,
    rel err ~1e-5) so the exp-bound inner loop is paced by neither alone
  - outU = v'_h.T @ expT  ([65 x 512] in PSUM, row 64 = softmax row-sum)
  - normalize without any transpose: reciprocal of row 64 -> tiny
    partition-broadcast DMA [1,512]->[64,512] -> one tensor_mul writes the
    normalized e-major tile into the pair-stacked stk buffer
  - y = sum_pairs stk_pair.T @ Wo_pair  (K=128); the 4 qt-groups of block
    qb are interjected into block qb+1's kc loop (PE slack there), so the
    normalize->project->DMA chain never stalls the exp pipeline
"""

import sys

for _p in ("/opt/trn_rl_repo", "/root/.axon_site/_ro/trn_rl_repo"):
    if _p not in sys.path:
        sys.path.insert(0, _p)

import numpy as np
import ml_dtypes

import concourse.bass as bass
import concourse.tile as tile
from concourse import bacc, mybir
from concourse.bass_utils import run_bass_kernel_spmd

BF16 = mybir.dt.bfloat16
F32 = mybir.dt.float32
AF = mybir.ActivationFunctionType

B, S, D, H, HD = 2, 2048, 1024, 16, 64
NCORES = 8
HPC = 4          # heads per core
NPAIR = 2        # head pairs per core
DC = D // 128    # 8 d-chunks
KC = S // 128    # 16 k-chunks
NQB = 4          # q blocks of 512
QB = 512
NQT = QB // 128  # q tiles per block

# kc iterations whose h2=1 exp half runs on the vector engine (must avoid
# 0 and 14/15: the deferred attn@v matmul would break the PSUM
# accumulation-group start/stop ordering). qb0-p0 runs everything on the
# scalar engine: that stretch is PE-bound (interjected projections) and its
# interjects keep the vector engine busy with PSUM drains already.
DVE_KCS_QB0_P0 = frozenset()
DVE_KCS_QB0_P1 = frozenset({2, 6, 10, 13})
DVE_KCS_STEADY = frozenset({2, 5, 8, 11, 13})

_CACHE = {}
_EXP_OPS = {}


def _register_exp_ops():
    """Register two custom DVE ops computing exp(x*scale) as
    (1 + t + t^2/2 + t^3/6)^64 with t = x*scale/64 (rel err ~1.3e-5):
    ANT_EXPP_SEED evaluates the cubic, ANT_EXPP_SQ6 squares six times."""
    if _EXP_OPS:
        return
    import concourse.dve_ops as dops
    from concourse.dve_spec import Spec, Src0, C0, C1, C2, One, sq
    from concourse.dve_spec import lower as dve_lower
    from concourse.dve_uop import DveOpSpec

    t = Src0 * C0
    t2 = t * t
    t3 = t2 * t
    seed_spec = Spec(
        body=((One + t) + t2 * C1) + t3 * C2,
        reference=lambda in0, in1, s0, s1, imm2: (
            1.0 + in0 * s0 + (in0 * s0) ** 2 * s1 + (in0 * s0) ** 3 * imm2
        ).astype(np.float32),
    )
    b = Src0
    for _ in range(6):
        b = sq(b)
    sq6_spec = Spec(
        body=b,
        reference=lambda in0, in1, s0, s1, imm2: (
            in0.astype(np.float64) ** 64
        ).astype(np.float32),
    )

    for name, spec in (("ANT_EXPP_SEED", seed_spec), ("ANT_EXPP_SQ6", sq6_spec)):
        if name not in dops._SUB_OPCODE_FOR_NAME:
            row = max(dops._SUB_OPCODE_FOR_NAME.values()) + 1
            assert row < 0x20, "custom-DVE row field overflow"
            dops._SUB_OPCODE_FOR_NAME[name] = row
        shas = {}
        for ver in ("v3", "v4"):
            uops = dve_lower(spec, ver=ver)
            shas[ver] = DveOpSpec(
                name=name, opcode=dops._SUB_OPCODE_FOR_NAME[name], uops=uops,
                rd1_en=False,
            ).sha(ver)
        op = dops.DveOp(name, spec, subdim=False, uops_sha=shas)
        if all(o.name != name for o in dops.OPS):
            dops.OPS.append(op)
        dops.CUSTOM_DVE_SPECS[name] = spec
        _EXP_OPS[name] = op


def _build_program():
    _register_exp_ops()
    seed_op = _EXP_OPS["ANT_EXPP_SEED"]
    sq6_op = _EXP_OPS["ANT_EXPP_SQ6"]

    nc = bacc.Bacc("TRN2", target_bir_lowering=False, debug=False, num_devices=NCORES)

    xt = nc.dram_tensor("xt", [D, S], BF16, kind="ExternalInput").ap()
    et = nc.dram_tensor("et", [D, S], BF16, kind="ExternalInput").ap()
    wq = nc.dram_tensor("wq", [128, NPAIR, DC, 128], BF16, kind="ExternalInput").ap()
    wk = nc.dram_tensor("wk", [128, NPAIR, DC, 128], BF16, kind="ExternalInput").ap()
    wv = nc.dram_tensor("wv", [128, DC, 256], BF16, kind="ExternalInput").ap()
    wo = nc.dram_tensor("wo", [128, NPAIR, D], BF16, kind="ExternalInput").ap()
    bq = nc.dram_tensor("bq", [128, NPAIR], F32, kind="ExternalInput").ap()
    bk = nc.dram_tensor("bk", [128, NPAIR], F32, kind="ExternalInput").ap()
    # bf16 partials: host sums 4 partials per batch in f32; the bf16
    # rounding of the partials adds ~1.6e-3 rel err (vs 2e-2 budget) and
    # halves the output DMA traffic
    out = nc.dram_tensor("out", [S, D], BF16, kind="ExternalOutput").ap()

    with tile.TileContext(nc) as tc:
        from contextlib import ExitStack

        with ExitStack() as ctx:
            wts = ctx.enter_context(tc.tile_pool(name="wts", bufs=1))
            big = ctx.enter_context(tc.tile_pool(name="big", bufs=1))

            # weights + activations: the first (critical) activation block is
            # spread over the three DMA-capable queues (sync, scalar, gpsimd)
            # so the prologue's DMA critical path is ~3x shorter; wq/wk go
            # first on gpsimd since the first k projection needs them
            wq_sb = wts.tile([128, NPAIR, DC, 128], BF16, name="wq_sb")
            wk_sb = wts.tile([128, NPAIR, DC, 128], BF16, name="wk_sb")
            wv_sb = wts.tile([128, DC, 256], BF16, name="wv_sb")
            wo_sb = wts.tile([128, NPAIR, D], BF16, name="wo_sb")
            bq_sb = wts.tile([128, NPAIR], F32, name="bq_sb")
            bk_sb = wts.tile([128, NPAIR], F32, name="bk_sb")
            nc.gpsimd.dma_start(wk_sb, wk)
            nc.gpsimd.dma_start(wq_sb, wq)
            nc.gpsimd.dma_start(bk_sb, bk)
            nc.gpsimd.dma_start(bq_sb, bq)

            xT = [big.tile([128, S], BF16, name=f"xT{d}") for d in range(DC)]
            eT = [big.tile([128, S], BF16, name=f"eT{d}") for d in range(DC)]
            qs = [nc.sync, nc.sync, nc.sync, nc.scalar, nc.scalar, nc.scalar,
                  nc.gpsimd, nc.gpsimd]
            sl0 = slice(0, QB)
            for d in range(DC):
                qs[d].dma_start(eT[d][:, sl0], et[d * 128:(d + 1) * 128, sl0])
            for d in range(DC):
                qs[d].dma_start(xT[d][:, sl0], xt[d * 128:(d + 1) * 128, sl0])
            nc.gpsimd.dma_start(wv_sb, wv)
            nc.gpsimd.dma_start(wo_sb, wo)
            for sb4 in range(1, NQB):
                sl = slice(sb4 * QB, (sb4 + 1) * QB)
                for d in range(DC):
                    nc.sync.dma_start(eT[d][:, sl], et[d * 128:(d + 1) * 128, sl])
            for sb4 in range(1, NQB):
                sl = slice(sb4 * QB, (sb4 + 1) * QB)
                for d in range(DC):
                    nc.sync.dma_start(xT[d][:, sl], xt[d * 128:(d + 1) * 128, sl])

            # ---- unified PSUM pools (8 banks total, live for whole kernel) ----
            dpool = ctx.enter_context(tc.tile_pool(name="dpool", bufs=4,
                                                   space="DRAM"))
            psc = ctx.enter_context(tc.tile_pool(name="psc", bufs=2, space="PSUM"))
            pou = ctx.enter_context(tc.tile_pool(name="pou", bufs=2, space="PSUM"))
            py = ctx.enter_context(tc.tile_pool(name="py", bufs=2, space="PSUM"))
            wk2 = ctx.enter_context(tc.tile_pool(name="wk2", bufs=2))
            expp = ctx.enter_context(tc.tile_pool(name="expp", bufs=6))
            exps = ctx.enter_context(tc.tile_pool(name="exps", bufs=2))

            # ---- projections; only kT[0] + v gate the first attention ----
            qT = [big.tile([128, S], BF16, name=f"qT{p}") for p in range(NPAIR)]
            kT = [big.tile([128, S], BF16, name=f"kT{p}") for p in range(NPAIR)]
            v = [big.tile([128, HPC, 65], BF16, name=f"v{s}") for s in range(KC)]

            def project_k_chunk(p, sb4):
                # deferred projections use the y-slot (idle during attention)
                sl = slice(sb4 * QB, (sb4 + 1) * QB)
                pk = py.tile([128, QB], F32, name="pk", tag="y")
                for d in range(DC):
                    nc.tensor.matmul(pk, wk_sb[:, p, d, :], eT[d][:, sl],
                                     start=(d == 0), stop=(d == DC - 1))
                nc.vector.tensor_scalar_add(kT[p][:, sl], pk, bk_sb[:, p:p + 1])

            def project_q(p, qb, tag="sc"):
                qsl = slice(qb * QB, (qb + 1) * QB)
                pool = psc if tag == "sc" else py
                pq = pool.tile([128, QB], F32, name="pq", tag=tag)
                for d in range(DC):
                    nc.tensor.matmul(pq, wq_sb[:, p, d, :], xT[d][:, qsl],
                                     start=(d == 0), stop=(d == DC - 1))
                nc.vector.tensor_scalar_add(qT[p][:, qsl], pq, bq_sb[:, p:p + 1])

            def project_v_tile(s):
                pv = py.tile([128, 256], F32, name="pv", tag="y")
                for d in range(DC):
                    nc.tensor.matmul(pv, eT[d][:, s * 128:(s + 1) * 128],
                                     wv_sb[:, d, :],
                                     start=(d == 0), stop=(d == DC - 1))
                nc.vector.tensor_copy(
                    v[s][:, :, 0:64], pv.rearrange("p (h e) -> p h e", h=HPC))
                nc.gpsimd.memset(v[s][:, :, 64:65], 1.0)

            # out-projection for one 128-row q tile of block qbx; interjected
            # into the following block's kc loop (or emitted directly for the
            # last block)
            def outproj_qt(qbx, qt, stk_pair):
                tsl = slice(qt * 128, (qt + 1) * 128)
                ysb = wk2.tile([128, D], BF16, name="ysb", tag="ysb", bufs=2)
                for dc2 in range(2):
                    dsl = slice(dc2 * 512, (dc2 + 1) * 512)
                    yp = py.tile([128, 512], F32, name="yp", tag="y")
                    for p in range(NPAIR):
                        nc.tensor.matmul(yp, stk_pair[p][:, tsl],
                                         wo_sb[:, p, dsl],
                                         start=(p == 0), stop=(p == NPAIR - 1))
                    nc.vector.tensor_copy(ysb[:, dsl], yp)
                # alternate output DMAs between two engine queues
                eng = nc.sync if qt % 2 == 0 else nc.scalar
                eng.dma_start(out[qbx * QB + qt * 128:
                                  qbx * QB + (qt + 1) * 128, :], ysb)

            # PE warm-up: ~40 tiny matmuls on a zero tile run during the
            # input-DMA wait, so the HAM clock gate reaches 8/8 (2.4 GHz)
            # before the real projections start (saves ~5us of cold-clock
            # projection time)
            warm = wts.tile([128, 64], BF16, name="warm")
            nc.gpsimd.memset(warm, 0.0)
            wdum = py.tile([64, 64], F32, name="wdum", tag="y")
            for _ in range(40):
                nc.tensor.matmul(wdum, warm[:, 0:64], warm[:, 0:64])

            # minimal prologue: just what (qb0, pair0, kc0) needs; every other
            # projection is interjected into the first q block's attention
            # loops (PE slack there; exp is the pacing engine in the kc loop)
            project_k_chunk(0, 0)
            project_v_tile(0)
            project_q(0, 0)

            def interject_qb0_p0(kc):
                if kc == 0:
                    project_v_tile(1); project_v_tile(2)
                elif kc == 1:
                    project_v_tile(3); project_k_chunk(0, 1)
                elif kc == 2:
                    project_v_tile(4); project_v_tile(5)
                elif kc == 3:
                    project_v_tile(6); project_v_tile(7)
                elif kc == 5:
                    project_k_chunk(0, 2)
                elif kc == 6:
                    project_v_tile(8); project_v_tile(9)
                elif kc == 7:
                    project_v_tile(10); project_v_tile(11)
                elif kc == 9:
                    project_k_chunk(0, 3)
                elif kc == 10:
                    project_v_tile(12); project_v_tile(13)
                elif kc == 11:
                    project_v_tile(14); project_v_tile(15)
                elif kc == 12:
                    project_k_chunk(1, 0)
                elif kc == 13:
                    project_q(1, 0)
                elif kc == 15:
                    project_q(0, 1, tag="y")

            def interject_qb0_p1(kc):
                if kc == 0:
                    project_k_chunk(1, 1)
                elif kc == 4:
                    project_k_chunk(1, 2)
                elif kc == 8:
                    project_k_chunk(1, 3)
                elif kc == 11:
                    project_q(1, 1, tag="y")

            # ---- attention + (deferred) output projection ----
            prev_stk = None  # stk tiles of block qb-1, out-projected during qb
            for qb in range(NQB):
                qsl = slice(qb * QB, (qb + 1) * QB)
                stk = [wk2.tile([128, QB], BF16, name=f"stk{p}", tag=f"stk{p}",
                                bufs=2) for p in range(NPAIR)]
                for p in range(NPAIR):
                    if qb == 0:
                        interject = interject_qb0_p0 if p == 0 else interject_qb0_p1
                        dve_kcs = DVE_KCS_QB0_P0 if p == 0 else DVE_KCS_QB0_P1
                    else:
                        dve_kcs = DVE_KCS_STEADY

                        def interject(kc, p=p, qb=qb):
                            # previous block's out-projection rides the PE
                            # slack of the p=0 loop (slots chosen off the
                            # DVE-exp kcs); next block's q projection at kc=11
                            if p == 0 and kc in (3, 6, 9, 12):
                                outproj_qt(qb - 1, (kc - 3) // 3, prev_stk)
                            elif kc == 11 and qb < NQB - 1:
                                project_q(p, qb + 1, tag="y")
                    ou = [pou.tile([65, QB], F32, name=f"ou{h2}", tag="ou")
                          for h2 in range(2)]
                    pending = []  # deferred h2=1 attn@v of DVE-exp kcs
                    for kc in range(KC):
                        ksl = slice(kc * 128, (kc + 1) * 128)
                        sc = psc.tile([128, 2, QB], F32, name="sc", tag="sc")
                        ex = expp.tile([128, 2, QB], BF16, name="ex", tag="ex")
                        for h2 in range(2):
                            hp = slice(h2 * 64, (h2 + 1) * 64)
                            nc.tensor.matmul(sc[:, h2, :], kT[p][hp, ksl],
                                             qT[p][hp, qsl])
                        if kc in dve_kcs:
                            # split: scalar engine exps h2=0, vector engine
                            # exps h2=1 via the custom poly-squaring chain
                            nc.scalar.activation(ex[:, 0, :], sc[:, 0, :],
                                                 AF.Exp, scale=0.125)
                            sd = exps.tile([128, QB], F32, name="sd", tag="sd")
                            nc.vector._custom_dve(
                                seed_op, out=sd, in0=sc[:, 1, :],
                                s0=0.125 / 64.0, s1=0.5, imm2=1.0 / 6.0)
                            nc.vector._custom_dve(
                                sq6_op, out=ex[:, 1, :], in0=sd)
                        else:
                            nc.scalar.activation(ex, sc, AF.Exp, scale=0.125)
                        # flush deferred h2=1 attn@v two iterations late,
                        # AFTER this kc's scores: the PE FIFO would otherwise
                        # head-of-line block on the slower DVE exp chain
                        # (PSUM accumulation order within the group is free;
                        # only kc=0 start / kc=15 stop must stay in place)
                        while pending and pending[0][0] <= kc - 2:
                            pkc, pex = pending.pop(0)
                            nc.tensor.matmul(ou[1], v[pkc][:, 2 * p + 1, :],
                                             pex[:, 1, :],
                                             start=(pkc == 0),
                                             stop=(pkc == KC - 1))
                        nc.tensor.matmul(ou[0], v[kc][:, 2 * p, :],
                                         ex[:, 0, :],
                                         start=(kc == 0), stop=(kc == KC - 1))
                        if kc in dve_kcs:
                            pending.append((kc, ex))
                        else:
                            nc.tensor.matmul(ou[1], v[kc][:, 2 * p + 1, :],
                                             ex[:, 1, :],
                                             start=(kc == 0), stop=(kc == KC - 1))
                        if interject is not None:
                            interject(kc)
                    # dve_kcs <= 13 guarantees everything flushed before the
                    # kc=15 stop matmul
                    assert not pending
                    for h2 in range(2):
                        # copy PSUM->SBUF promptly so the ou slot frees for the
                        # next pair; normalize off the critical path:
                        # reciprocal of rowsum row -> partition-broadcast via a
                        # DRAM bounce (step-0 partition APs are DRAM-only) ->
                        # one multiply into the pair-stacked e-major tile
                        osb = wk2.tile([65, QB], F32, name=f"osb{h2}",
                                       tag=f"osb{h2}", bufs=2)
                        nc.vector.tensor_copy(osb, ou[h2])
                        rr = wk2.tile([65, QB], F32, name="rr", tag="rr", bufs=4)
                        nc.vector.reciprocal_approx_fast(rr, osb)
                        rrd = dpool.tile([1, QB], F32, name="rrd", tag="rrd")
                        nc.gpsimd.dma_start(rrd, rr[64:65, :])
                        rb = wk2.tile([64, QB], F32, name="rb", tag="rb", bufs=4)
                        rr_bcast = bass.AP(tensor=rrd.tensor, offset=rrd.offset,
                                           ap=[[0, 64]] + list(rrd.ap[1:]))
                        nc.gpsimd.dma_start(rb, rr_bcast)
                        nc.gpsimd.tensor_mul(stk[p][h2 * 64:(h2 + 1) * 64, :],
                                             osb[0:64, :], rb)
                prev_stk = stk
            # final block's out-projection has no following loop to hide in
            for qt in range(NQT):
                outproj_qt(NQB - 1, qt, prev_stk)

    nc.compile()
    return nc


def _bf16(a):
    return np.ascontiguousarray(a.astype(ml_dtypes.bfloat16))


def _host_prep(inputs):
    x = np.asarray(inputs["x"], np.float32)
    enc = np.asarray(inputs["encoder_output"], np.float32)
    Wq = np.asarray(inputs["Wq"], np.float32)
    bq = np.asarray(inputs["bq"], np.float32)
    Wk = np.asarray(inputs["Wk"], np.float32)
    bk = np.asarray(inputs["bk"], np.float32)
    Wv = np.asarray(inputs["Wv"], np.float32)
    Wo = np.asarray(inputs["Wo"], np.float32)

    xt_b = [_bf16(x[b].T) for b in range(B)]
    et_b = [_bf16(enc[b].T) for b in range(B)]

    in_maps = []
    for c in range(NCORES):
        b = c // 4
        hb = HPC * (c % 4)

        wq_c = Wq[hb:hb + 4].reshape(2, 2, DC, 128, HD)  # [pair, hw, dc, dp, e]
        wq_c = wq_c.transpose(3, 0, 2, 1, 4).reshape(128, NPAIR, DC, 128)
        wk_c = Wk[hb:hb + 4].reshape(2, 2, DC, 128, HD)
        wk_c = wk_c.transpose(3, 0, 2, 1, 4).reshape(128, NPAIR, DC, 128)
        wv_c = Wv[hb:hb + 4].reshape(4, DC, 128, HD)
        wv_c = wv_c.transpose(2, 1, 0, 3).reshape(128, DC, 256)
        wo_c = Wo[hb * HD:(hb + 4) * HD].reshape(2, 2, HD, D)  # [pair, hw, e, d]
        wo_c = wo_c.transpose(1, 2, 0, 3).reshape(128, NPAIR, D)
        bq_c = bq[hb:hb + 4].reshape(2, 2, HD).transpose(1, 2, 0).reshape(128, NPAIR)
        bk_c = bk[hb:hb + 4].reshape(2, 2, HD).transpose(1, 2, 0).reshape(128, NPAIR)

        in_maps.append({
            "xt": xt_b[b],
            "et": et_b[b],
            "wq": _bf16(wq_c),
            "wk": _bf16(wk_c),
            "wv": _bf16(wv_c),
            "wo": _bf16(wo_c),
            "bq": np.ascontiguousarray(bq_c),
            "bk": np.ascontiguousarray(bk_c),
        })
    return in_maps


def kernel(**inputs):
    if "nc" not in _CACHE:
        _CACHE["nc"] = _build_program()
    nc = _CACHE["nc"]

    in_maps = _host_prep(inputs)
    res = None
    for attempt in range(3):
        try:
            res = run_bass_kernel_spmd(nc, in_maps, core_ids=list(range(NCORES)))
            break
        except Exception:
            if attempt == 2:
                raise
            import time
            time.sleep(5)
    _CACHE["last_results"] = res

    bv = np.asarray(inputs["bv"], np.float32)
    Wo = np.asarray(inputs["Wo"], np.float32)
    bo = np.asarray(inputs["bo"], np.float32)
    const_d = bo + np.einsum("he,hed->d", bv,
                             Wo.reshape(H, HD, D)).astype(np.float32)

    out = np.empty((B, S, D), np.float32)
    for b in range(B):
        acc = res.results[4 * b]["out"].astype(np.float32).copy()
        for c in range(4 * b + 1, 4 * b + 4):
            acc += res.results[c]["out"]
        out[b] = acc + const_d
    return out


# revision 22
# speedup vs baseline: 1.1372x; 1.0103x over previous
"""Trainium2 Bass kernel for CrossMultiHeadedSelfAttention.

Problem: B=2, SQ=SK=2048, D=1024, H=16, HD=64 cross-attention
  q = x @ Wq + bq ; k = enc @ Wk + bk ; v = enc @ Wv + bv   (per head)
  out = softmax(q k^T / sqrt(HD)) v  -> concat heads -> @ Wo + bo

Sharding: 8 cores = 2 batches x 4 head-groups (4 heads per core).
Each core computes a partial output projection over its 4 heads; the host
sums the 4 partials per batch and adds the constant term
(bo + sum_h bv_h @ Wo_h, exact because softmax rows sum to 1).

Device-side math (per core, bf16 matmuls, f32 accumulation):
  - x/enc are pre-transposed AND pre-cast to bf16 on the host, so xT/encT
    d-major tiles load with fully contiguous DMA
  - qT/kT in [head-pair e (128) x seq] layout, bias via per-partition
    tensor_scalar add; v in natural [s, 4*65] layout with a ones column
    per head (gives softmax row-sums for free in the attn@v matmul)
  - scoresT chunk: the two heads of a pair are row-tile-packed on the PE
    (tile_position (0,0)/(64,0)) and run concurrently
  - exp is split between the scalar engine (table exp) and the vector
    engine (custom 2-op chain: cubic seed of e^{x/64} then 6 squarings,
    rel err ~1e-5) so the exp-bound inner loop is paced by neither alone
  - outU = v'_h.T @ expT  ([65 x 512] in PSUM, row 64 = softmax row-sum)
  - normalize without any transpose: reciprocal of row 64 -> tiny
    partition-broadcast DMA [1,512]->[64,512] -> one tensor_mul writes the
    normalized e-major tile into the pair-stacked stk buffer
  - y = sum_pairs stk_pair.T @ Wo_pair  (K=128); the 4 qt-groups of block
    qb are interjected into block qb+1's kc loop (PE slack there), so the
    normalize->project->DMA chain never stalls the exp pipeline
"""

import sys

for _p in ("/opt/trn_rl_repo", "/root/.axon_site/_ro/trn_rl_repo"):
    if _p not in sys.path:
        sys.path.insert(0, _p)

import numpy as np
import ml_dtypes

import concourse.bass as bass
import concourse.tile as tile
from concourse import bacc, mybir
from concourse.bass_utils import run_bass_kernel_spmd

BF16 = mybir.dt.bfloat16
F32 = mybir.dt.float32
AF = mybir.ActivationFunctionType

B, S, D, H, HD = 2, 2048, 1024, 16, 64
NCORES = 8
HPC = 4          # heads per core
NPAIR = 2        # head pairs per core
DC = D // 128    # 8 d-chunks
KC = S // 128    # 16 k-chunks
NQB = 4          # q blocks of 512
QB = 512
NQT = QB // 128  # q tiles per block

# kc iterations whose h2=1 exp half runs on the vector engine (must avoid
# 0 and 14/15: the deferred attn@v matmul would break the PSUM
# accumulation-group start/stop ordering). qb0-p0 runs everything on the
# scalar engine: that stretch is PE-bound (interjected projections) and its
# interjects keep the vector engine busy with PSUM drains already.
DVE_KCS_QB0_P0 = frozenset()
DVE_KCS_QB0_P1 = frozenset({2, 6, 10, 13})
DVE_KCS_STEADY_P0 = frozenset({3, 6, 9, 12})
DVE_KCS_STEADY_P1 = frozenset({2, 5, 8, 11, 13})

_CACHE = {}
_EXP_OPS = {}


def _register_exp_ops():
    """Register two custom DVE ops computing exp(x*scale) as
    (1 + t + t^2/2 + t^3/6)^64 with t = x*scale/64 (rel err ~1.3e-5):
    ANT_EXPP_SEED evaluates the cubic, ANT_EXPP_SQ6 squares six times."""
    if _EXP_OPS:
        return
    import concourse.dve_ops as dops
    from concourse.dve_spec import Spec, Src0, C0, C1, C2, One, sq
    from concourse.dve_spec import lower as dve_lower
    from concourse.dve_uop import DveOpSpec

    t = Src0 * C0
    t2 = t * t
    t3 = t2 * t
    seed_spec = Spec(
        body=((One + t) + t2 * C1) + t3 * C2,
        reference=lambda in0, in1, s0, s1, imm2: (
            1.0 + in0 * s0 + (in0 * s0) ** 2 * s1 + (in0 * s0) ** 3 * imm2
        ).astype(np.float32),
    )
    b = Src0
    for _ in range(6):
        b = sq(b)
    sq6_spec = Spec(
        body=b,
        reference=lambda in0, in1, s0, s1, imm2: (
            in0.astype(np.float64) ** 64
        ).astype(np.float32),
    )

    for name, spec in (("ANT_EXPP_SEED", seed_spec), ("ANT_EXPP_SQ6", sq6_spec)):
        if name not in dops._SUB_OPCODE_FOR_NAME:
            row = max(dops._SUB_OPCODE_FOR_NAME.values()) + 1
            assert row < 0x20, "custom-DVE row field overflow"
            dops._SUB_OPCODE_FOR_NAME[name] = row
        shas = {}
        for ver in ("v3", "v4"):
            uops = dve_lower(spec, ver=ver)
            shas[ver] = DveOpSpec(
                name=name, opcode=dops._SUB_OPCODE_FOR_NAME[name], uops=uops,
                rd1_en=False,
            ).sha(ver)
        op = dops.DveOp(name, spec, subdim=False, uops_sha=shas)
        if all(o.name != name for o in dops.OPS):
            dops.OPS.append(op)
        dops.CUSTOM_DVE_SPECS[name] = spec
        _EXP_OPS[name] = op


def _build_program():
    _register_exp_ops()
    seed_op = _EXP_OPS["ANT_EXPP_SEED"]
    sq6_op = _EXP_OPS["ANT_EXPP_SQ6"]

    nc = bacc.Bacc("TRN2", target_bir_lowering=False, debug=False, num_devices=NCORES)

    xt = nc.dram_tensor("xt", [D, S], BF16, kind="ExternalInput").ap()
    et = nc.dram_tensor("et", [D, S], BF16, kind="ExternalInput").ap()
    wq = nc.dram_tensor("wq", [128, NPAIR, DC, 128], BF16, kind="ExternalInput").ap()
    wk = nc.dram_tensor("wk", [128, NPAIR, DC, 128], BF16, kind="ExternalInput").ap()
    wv = nc.dram_tensor("wv", [128, DC, 256], BF16, kind="ExternalInput").ap()
    wo = nc.dram_tensor("wo", [128, NPAIR, D], BF16, kind="ExternalInput").ap()
    bq = nc.dram_tensor("bq", [128, NPAIR], F32, kind="ExternalInput").ap()
    bk = nc.dram_tensor("bk", [128, NPAIR], F32, kind="ExternalInput").ap()
    # bf16 partials: host sums 4 partials per batch in f32; the bf16
    # rounding of the partials adds ~1.6e-3 rel err (vs 2e-2 budget) and
    # halves the output DMA traffic
    out = nc.dram_tensor("out", [S, D], BF16, kind="ExternalOutput").ap()

    with tile.TileContext(nc) as tc:
        from contextlib import ExitStack

        with ExitStack() as ctx:
            wts = ctx.enter_context(tc.tile_pool(name="wts", bufs=1))
            big = ctx.enter_context(tc.tile_pool(name="big", bufs=1))

            # weights + activations: the first (critical) activation block is
            # spread over the three DMA-capable queues (sync, scalar, gpsimd)
            # so the prologue's DMA critical path is ~3x shorter; wq/wk go
            # first on gpsimd since the first k projection needs them
            wq_sb = wts.tile([128, NPAIR, DC, 128], BF16, name="wq_sb")
            wk_sb = wts.tile([128, NPAIR, DC, 128], BF16, name="wk_sb")
            wv_sb = wts.tile([128, DC, 256], BF16, name="wv_sb")
            wo_sb = wts.tile([128, NPAIR, D], BF16, name="wo_sb")
            bq_sb = wts.tile([128, NPAIR], F32, name="bq_sb")
            bk_sb = wts.tile([128, NPAIR], F32, name="bk_sb")
            nc.gpsimd.dma_start(wk_sb, wk)

            xT = [big.tile([128, S], BF16, name=f"xT{d}") for d in range(DC)]
            eT = [big.tile([128, S], BF16, name=f"eT{d}") for d in range(DC)]
            qs = [nc.sync, nc.sync, nc.sync, nc.scalar, nc.scalar, nc.scalar,
                  nc.gpsimd, nc.gpsimd]
            sl0 = slice(0, QB)
            for d in range(DC):
                qs[d].dma_start(eT[d][:, sl0], et[d * 128:(d + 1) * 128, sl0])
            nc.gpsimd.dma_start(wq_sb, wq)
            for d in range(DC):
                qs[d].dma_start(xT[d][:, sl0], xt[d * 128:(d + 1) * 128, sl0])
            nc.gpsimd.dma_start(bk_sb, bk)
            nc.gpsimd.dma_start(bq_sb, bq)
            nc.gpsimd.dma_start(wv_sb, wv)
            nc.gpsimd.dma_start(wo_sb, wo)
            for sb4 in range(1, NQB):
                sl = slice(sb4 * QB, (sb4 + 1) * QB)
                for d in range(DC):
                    nc.sync.dma_start(eT[d][:, sl], et[d * 128:(d + 1) * 128, sl])
            for sb4 in range(1, NQB):
                sl = slice(sb4 * QB, (sb4 + 1) * QB)
                for d in range(DC):
                    nc.sync.dma_start(xT[d][:, sl], xt[d * 128:(d + 1) * 128, sl])

            # ---- unified PSUM pools (8 banks total, live for whole kernel) ----
            dpool = ctx.enter_context(tc.tile_pool(name="dpool", bufs=4,
                                                   space="DRAM"))
            psc = ctx.enter_context(tc.tile_pool(name="psc", bufs=2, space="PSUM"))
            pou = ctx.enter_context(tc.tile_pool(name="pou", bufs=2, space="PSUM"))
            py = ctx.enter_context(tc.tile_pool(name="py", bufs=2, space="PSUM"))
            wk2 = ctx.enter_context(tc.tile_pool(name="wk2", bufs=2))
            expp = ctx.enter_context(tc.tile_pool(name="expp", bufs=6))
            exps = ctx.enter_context(tc.tile_pool(name="exps", bufs=2))

            # ---- projections; only kT[0] + v gate the first attention ----
            qT = [big.tile([128, S], BF16, name=f"qT{p}") for p in range(NPAIR)]
            kT = [big.tile([128, S], BF16, name=f"kT{p}") for p in range(NPAIR)]
            v = [big.tile([128, HPC, 65], BF16, name=f"v{s}") for s in range(KC)]

            def project_k_chunk(p, sb4):
                # deferred projections use the y-slot (idle during attention)
                sl = slice(sb4 * QB, (sb4 + 1) * QB)
                pk = py.tile([128, QB], F32, name="pk", tag="y")
                for d in range(DC):
                    nc.tensor.matmul(pk, wk_sb[:, p, d, :], eT[d][:, sl],
                                     start=(d == 0), stop=(d == DC - 1))
                nc.vector.tensor_scalar_add(kT[p][:, sl], pk, bk_sb[:, p:p + 1])

            def project_q(p, qb, tag="sc"):
                qsl = slice(qb * QB, (qb + 1) * QB)
                pool = psc if tag == "sc" else py
                pq = pool.tile([128, QB], F32, name="pq", tag=tag)
                for d in range(DC):
                    nc.tensor.matmul(pq, wq_sb[:, p, d, :], xT[d][:, qsl],
                                     start=(d == 0), stop=(d == DC - 1))
                nc.vector.tensor_scalar_add(qT[p][:, qsl], pq, bq_sb[:, p:p + 1])

            def project_v_tile(s):
                pv = py.tile([128, 256], F32, name="pv", tag="y")
                for d in range(DC):
                    nc.tensor.matmul(pv, eT[d][:, s * 128:(s + 1) * 128],
                                     wv_sb[:, d, :],
                                     start=(d == 0), stop=(d == DC - 1))
                nc.vector.tensor_copy(
                    v[s][:, :, 0:64], pv.rearrange("p (h e) -> p h e", h=HPC))
                nc.gpsimd.memset(v[s][:, :, 64:65], 1.0)

            # --- spread-emission helpers -------------------------------------
            # A full 8-matmul projection group bunched into one kc slot makes
            # the PE locally binding (PE and ACT totals are within ~15% of
            # each other in steady state) and starves the exp pipeline, so
            # deferred work is emitted at ~1 matmul per kc iteration.

            def spread_kproj(tasks, p_t, sb4, kcs):
                cell = {}
                sl = slice(sb4 * QB, (sb4 + 1) * QB)

                def mk(d):
                    def go():
                        if d == 0:
                            cell["t"] = py.tile([128, QB], F32, name="pk",
                                                tag="y")
                        nc.tensor.matmul(cell["t"], wk_sb[:, p_t, d, :],
                                         eT[d][:, sl],
                                         start=(d == 0), stop=(d == DC - 1))
                        if d == DC - 1:
                            nc.vector.tensor_scalar_add(
                                kT[p_t][:, sl], cell["t"], bk_sb[:, p_t:p_t + 1])
                    return go
                n = len(kcs)
                for i in range(DC):
                    tasks[kcs[i * n // DC]].append(mk(i))

            def spread_qproj(tasks, p_t, blk, kcs):
                cell = {}
                sl = slice(blk * QB, (blk + 1) * QB)

                def mk(d):
                    def go():
                        if d == 0:
                            cell["t"] = py.tile([128, QB], F32, name="pq",
                                                tag="y")
                        nc.tensor.matmul(cell["t"], wq_sb[:, p_t, d, :],
                                         xT[d][:, sl],
                                         start=(d == 0), stop=(d == DC - 1))
                        if d == DC - 1:
                            nc.vector.tensor_scalar_add(
                                qT[p_t][:, sl], cell["t"], bq_sb[:, p_t:p_t + 1])
                    return go
                n = len(kcs)
                for i in range(DC):
                    tasks[kcs[i * n // DC]].append(mk(i))

            # out-projection of block qbx, spread one matmul per kc over the
            # following block's p=0 loop (16 matmuls, 8 copies, 4 DMAs).
            # Group G=(qt,dc2) runs its two pair-matmuls at kcs 2G/2G+1, its
            # PSUM->SBUF copy at kc 2G+2 (G=7's copy + the last DMA land just
            # after the loop), the row DMA after the dc2=1 copy.
            def build_outproj(tasks, qbx, stk_pair):
                st = {}

                def mk_mm(i):
                    G, pp = divmod(i, 2)
                    qt, dc2 = divmod(G, 2)
                    dsl = slice(dc2 * 512, (dc2 + 1) * 512)
                    tsl = slice(qt * 128, (qt + 1) * 128)

                    def go():
                        if pp == 0:
                            st[G] = py.tile([128, 512], F32, name="yp", tag="y")
                        nc.tensor.matmul(st[G], stk_pair[pp][:, tsl],
                                         wo_sb[:, pp, dsl],
                                         start=(pp == 0), stop=(pp == 1))
                    return go

                def mk_copy(G):
                    qt, dc2 = divmod(G, 2)
                    dsl = slice(dc2 * 512, (dc2 + 1) * 512)

                    def go():
                        if dc2 == 0:
                            st[("ysb", qt)] = wk2.tile([128, D], BF16,
                                                       name="ysb", tag="ysb",
                                                       bufs=2)
                        nc.vector.tensor_copy(st[("ysb", qt)][:, dsl], st[G])
                        if dc2 == 1:
                            eng = nc.sync if qt % 2 == 0 else nc.scalar
                            eng.dma_start(
                                out[qbx * QB + qt * 128:
                                    qbx * QB + (qt + 1) * 128, :],
                                st[("ysb", qt)])
                    return go

                for i in range(16):
                    tasks[i].append(mk_mm(i))
                for G in range(8):
                    tasks[2 * G + 2].append(mk_copy(G))

            # PE warm-up: ~40 tiny matmuls on a zero tile run during the
            # input-DMA wait, so the HAM clock gate reaches 8/8 (2.4 GHz)
            # before the real projections start (saves ~5us of cold-clock
            # projection time). memset on the (idle) vector queue so the
            # dummies are runnable immediately.
            warm = wts.tile([128, 64], BF16, name="warm")
            nc.vector.memset(warm, 0.0)
            wdum = py.tile([64, 64], F32, name="wdum", tag="y")
            for _ in range(40):
                nc.tensor.matmul(wdum, warm[:, 0:64], warm[:, 0:64])

            # minimal prologue: just what (qb0, pair0, kc0) needs; every other
            # projection is interjected into the first q block's attention
            # loops (PE slack there; exp is the pacing engine in the kc loop)
            project_k_chunk(0, 0)
            project_v_tile(0)
            project_q(0, 0)

            def build_qb0_p0(tasks):
                from functools import partial
                sched = {0: [partial(project_v_tile, 1), partial(project_v_tile, 2)],
                         1: [partial(project_v_tile, 3),
                             partial(project_k_chunk, 0, 1)],
                         2: [partial(project_v_tile, 4), partial(project_v_tile, 5)],
                         3: [partial(project_v_tile, 6), partial(project_v_tile, 7)],
                         5: [partial(project_k_chunk, 0, 2)],
                         6: [partial(project_v_tile, 8), partial(project_v_tile, 9)],
                         7: [partial(project_v_tile, 10), partial(project_v_tile, 11)],
                         9: [partial(project_k_chunk, 0, 3)],
                         10: [partial(project_v_tile, 12), partial(project_v_tile, 13)],
                         11: [partial(project_v_tile, 14), partial(project_v_tile, 15)],
                         12: [partial(project_k_chunk, 1, 0)],
                         13: [partial(project_q, 1, 0)]}
                for kc, fns in sched.items():
                    tasks[kc].extend(fns)

            def build_qb0_p1(tasks):
                spread_kproj(tasks, 1, 1, [0, 1, 2])
                spread_kproj(tasks, 1, 2, [3, 4, 5])
                spread_kproj(tasks, 1, 3, [6, 7, 8])
                spread_qproj(tasks, 0, 1, [9, 10, 11])
                spread_qproj(tasks, 1, 1, [12, 13, 14])

            # ---- attention + (deferred) output projection ----
            from collections import defaultdict
            prev_stk = None  # stk tiles of block qb-1, out-projected during qb
            for qb in range(NQB):
                qsl = slice(qb * QB, (qb + 1) * QB)
                stk = [wk2.tile([128, QB], BF16, name=f"stk{p}", tag=f"stk{p}",
                                bufs=2) for p in range(NPAIR)]
                for p in range(NPAIR):
                    tasks = defaultdict(list)
                    if qb == 0:
                        if p == 0:
                            build_qb0_p0(tasks)
                            dve_kcs = DVE_KCS_QB0_P0
                        else:
                            build_qb0_p1(tasks)
                            dve_kcs = DVE_KCS_QB0_P1
                    elif p == 0:
                        build_outproj(tasks, qb - 1, prev_stk)
                        dve_kcs = DVE_KCS_STEADY_P0
                    else:
                        if qb < NQB - 1:
                            # next block's q projections, one matmul per kc
                            spread_qproj(tasks, 0, qb + 1, list(range(0, 8)))
                            spread_qproj(tasks, 1, qb + 1, list(range(8, 16)))
                        dve_kcs = DVE_KCS_STEADY_P1

                    def interject(kc):
                        for fn in tasks.get(kc, ()):
                            fn()
                    ou = [pou.tile([65, QB], F32, name=f"ou{h2}", tag="ou")
                          for h2 in range(2)]
                    pending = []  # deferred h2=1 attn@v of DVE-exp kcs
                    for kc in range(KC):
                        ksl = slice(kc * 128, (kc + 1) * 128)
                        sc = psc.tile([128, 2, QB], F32, name="sc", tag="sc")
                        ex = expp.tile([128, 2, QB], BF16, name="ex", tag="ex")
                        for h2 in range(2):
                            hp = slice(h2 * 64, (h2 + 1) * 64)
                            nc.tensor.matmul(sc[:, h2, :], kT[p][hp, ksl],
                                             qT[p][hp, qsl])
                        if kc in dve_kcs:
                            # split: scalar engine exps h2=0, vector engine
                            # exps h2=1 via the custom poly-squaring chain
                            nc.scalar.activation(ex[:, 0, :], sc[:, 0, :],
                                                 AF.Exp, scale=0.125)
                            sd = exps.tile([128, QB], F32, name="sd", tag="sd")
                            nc.vector._custom_dve(
                                seed_op, out=sd, in0=sc[:, 1, :],
                                s0=0.125 / 64.0, s1=0.5, imm2=1.0 / 6.0)
                            nc.vector._custom_dve(
                                sq6_op, out=ex[:, 1, :], in0=sd)
                        else:
                            nc.scalar.activation(ex, sc, AF.Exp, scale=0.125)
                        # flush deferred h2=1 attn@v two iterations late,
                        # AFTER this kc's scores: the PE FIFO would otherwise
                        # head-of-line block on the slower DVE exp chain
                        # (PSUM accumulation order within the group is free;
                        # only kc=0 start / kc=15 stop must stay in place)
                        while pending and pending[0][0] <= kc - 2:
                            pkc, pex = pending.pop(0)
                            nc.tensor.matmul(ou[1], v[pkc][:, 2 * p + 1, :],
                                             pex[:, 1, :],
                                             start=(pkc == 0),
                                             stop=(pkc == KC - 1))
                        nc.tensor.matmul(ou[0], v[kc][:, 2 * p, :],
                                         ex[:, 0, :],
                                         start=(kc == 0), stop=(kc == KC - 1))
                        if kc in dve_kcs:
                            pending.append((kc, ex))
                        else:
                            nc.tensor.matmul(ou[1], v[kc][:, 2 * p + 1, :],
                                             ex[:, 1, :],
                                             start=(kc == 0), stop=(kc == KC - 1))
                        if interject is not None:
                            interject(kc)
                    # dve_kcs <= 13 guarantees everything flushed before the
                    # kc=15 stop matmul
                    assert not pending
                    # tasks scheduled past the loop end (last out-proj copy)
                    for kcx in sorted(k for k in tasks if k >= KC):
                        for fn in tasks[kcx]:
                            fn()
                    for h2 in range(2):
                        # copy PSUM->SBUF promptly so the ou slot frees for the
                        # next pair; normalize off the critical path:
                        # reciprocal of rowsum row -> partition-broadcast via a
                        # DRAM bounce (step-0 partition APs are DRAM-only) ->
                        # one multiply into the pair-stacked e-major tile
                        osb = wk2.tile([65, QB], F32, name=f"osb{h2}",
                                       tag=f"osb{h2}", bufs=2)
                        nc.vector.tensor_copy(osb, ou[h2])
                        rr = wk2.tile([65, QB], F32, name="rr", tag="rr", bufs=4)
                        nc.vector.reciprocal_approx_fast(rr, osb)
                        rrd = dpool.tile([1, QB], F32, name="rrd", tag="rrd")
                        nc.gpsimd.dma_start(rrd, rr[64:65, :])
                        rb = wk2.tile([64, QB], F32, name="rb", tag="rb", bufs=4)
                        rr_bcast = bass.AP(tensor=rrd.tensor, offset=rrd.offset,
                                           ap=[[0, 64]] + list(rrd.ap[1:]))
                        nc.gpsimd.dma_start(rb, rr_bcast)
                        nc.gpsimd.tensor_mul(stk[p][h2 * 64:(h2 + 1) * 64, :],
                                             osb[0:64, :], rb)
                prev_stk = stk
            # final block's out-projection has no following loop to hide in;
            # emit with fine-grained copy/DMA interleave to shorten the tail
            for qt in range(NQT):
                tsl = slice(qt * 128, (qt + 1) * 128)
                ysb = wk2.tile([128, D], BF16, name="ysb", tag="ysb", bufs=2)
                for dc2 in range(2):
                    dsl = slice(dc2 * 512, (dc2 + 1) * 512)
                    yp = py.tile([128, 512], F32, name="yp", tag="y")
                    for p in range(NPAIR):
                        nc.tensor.matmul(yp, prev_stk[p][:, tsl],
                                         wo_sb[:, p, dsl],
                                         start=(p == 0), stop=(p == NPAIR - 1))
                    nc.vector.tensor_copy(ysb[:, dsl], yp)
                    eng = nc.sync if (2 * qt + dc2) % 2 == 0 else nc.scalar
                    eng.dma_start(out[(NQB - 1) * QB + qt * 128:
                                      (NQB - 1) * QB + (qt + 1) * 128, dsl],
                                  ysb[:, dsl])

    nc.compile()
    return nc


def _bf16(a):
    return np.ascontiguousarray(a.astype(ml_dtypes.bfloat16))


def _host_prep(inputs):
    x = np.asarray(inputs["x"], np.float32)
    enc = np.asarray(inputs["encoder_output"], np.float32)
    Wq = np.asarray(inputs["Wq"], np.float32)
    bq = np.asarray(inputs["bq"], np.float32)
    Wk = np.asarray(inputs["Wk"], np.float32)
    bk = np.asarray(inputs["bk"], np.float32)
    Wv = np.asarray(inputs["Wv"], np.float32)
    Wo = np.asarray(inputs["Wo"], np.float32)

    xt_b = [_bf16(x[b].T) for b in range(B)]
    et_b = [_bf16(enc[b].T) for b in range(B)]

    in_maps = []
    for c in range(NCORES):
        b = c // 4
        hb = HPC * (c % 4)

        wq_c = Wq[hb:hb + 4].reshape(2, 2, DC, 128, HD)  # [pair, hw, dc, dp, e]
        wq_c = wq_c.transpose(3, 0, 2, 1, 4).reshape(128, NPAIR, DC, 128)
        wk_c = Wk[hb:hb + 4].reshape(2, 2, DC, 128, HD)
        wk_c = wk_c.transpose(3, 0, 2, 1, 4).reshape(128, NPAIR, DC, 128)
        wv_c = Wv[hb:hb + 4].reshape(4, DC, 128, HD)
        wv_c = wv_c.transpose(2, 1, 0, 3).reshape(128, DC, 256)
        wo_c = Wo[hb * HD:(hb + 4) * HD].reshape(2, 2, HD, D)  # [pair, hw, e, d]
        wo_c = wo_c.transpose(1, 2, 0, 3).reshape(128, NPAIR, D)
        bq_c = bq[hb:hb + 4].reshape(2, 2, HD).transpose(1, 2, 0).reshape(128, NPAIR)
        bk_c = bk[hb:hb + 4].reshape(2, 2, HD).transpose(1, 2, 0).reshape(128, NPAIR)

        in_maps.append({
            "xt": xt_b[b],
            "et": et_b[b],
            "wq": _bf16(wq_c),
            "wk": _bf16(wk_c),
            "wv": _bf16(wv_c),
            "wo": _bf16(wo_c),
            "bq": np.ascontiguousarray(bq_c),
            "bk": np.ascontiguousarray(bk_c),
        })
    return in_maps


def kernel(**inputs):
    if "nc" not in _CACHE:
        _CACHE["nc"] = _build_program()
    nc = _CACHE["nc"]

    in_maps = _host_prep(inputs)
    res = None
    for attempt in range(3):
        try:
            res = run_bass_kernel_spmd(nc, in_maps, core_ids=list(range(NCORES)))
            break
        except Exception:
            if attempt == 2:
                raise
            import time
            time.sleep(5)
    _CACHE["last_results"] = res

    bv = np.asarray(inputs["bv"], np.float32)
    Wo = np.asarray(inputs["Wo"], np.float32)
    bo = np.asarray(inputs["bo"], np.float32)
    const_d = bo + np.einsum("he,hed->d", bv,
                             Wo.reshape(H, HD, D)).astype(np.float32)

    out = np.empty((B, S, D), np.float32)
    for b in range(B):
        acc = res.results[4 * b]["out"].astype(np.float32).copy()
        for c in range(4 * b + 1, 4 * b + 4):
            acc += res.results[c]["out"]
        out[b] = acc + const_d
    return out
